# revision 57
# baseline (speedup 1.0000x reference)
"""ARIMA(64, 1, 32) forecast kernel for Trainium2 (Bass/Tile).

Math: with D=1 differencing, the reference's full-series diff is dead code
except its last 64 values (the AR window), and the inverse-differencing
cumsum runs only over the 2048 predictions.  The output depends on
x[0, -65:, 0] plus the weights:

    d[j]  = xt[j+1] - xt[j]            (last 64 diffs = AR window)
    y_t   = sum_j a_j y_{t-j} + c      (AR(64), c = b_ar + b_ma, 2048 steps)
    out_n = x_last + sum_{t<=n+1} y_t

The sequential AR recurrence is parallelized on the tensor engine with the
65x65 augmented companion matrix C over the state s_t = [y_{t-63..t}, c]
(oldest first, constant lane carries c): s_t = C^t s_0.  Only the 32 states
t = 64,128,...,2048 are needed -- together they hold all 2048 predictions
in order.  They are computed by exponentiation-by-squaring (C^2..C^64=G,
then G^2..G^16) plus column doubling W_{2m} = [W_m | G^m W_m]; transposed
powers ride along via (A A)^T = A^T A^T, so no PE transposes are needed in
the chain.  The final cumsum is a triangular matmul whose 65-wide lhsT also
produces the chunk sums (row 64), a 32-element vector scan (exclusive chunk
offsets, x_last folded in), and a broadcast matmul accumulated on top of a
PE transpose so the result DMAs out contiguously.  All arithmetic is fp32
on device; the host only packs inputs into one DMA blob (layout, no math).

All 8 cores run the identical tiny kernel (the recurrence is replicated per
the sharding hint); core 0's output is returned.
"""

import numpy as np

import concourse.bacc as bacc
import concourse.mybir as mybir
import concourse.tile as tile
from concourse.bass_utils import run_bass_kernel_spmd

F32 = mybir.dt.float32
P = 64          # AR order = chunk size
NCHUNK = 32     # 2048 / 64
STEPS = 2048    # forecast horizon
N_CORES = 8
K = P + 1       # augmented state size

# blob column map (65 partitions x BLOB_F fp32)
C_COL = 0            # C skeleton  [0:65)
CT_COL = 65          # C^T skeleton [65:130)
XTA_COL = 130        # xt[1:65] in p0..63, b_ar at p64
XTB_COL = 131        # xt[0:64] in p0..63, -b_ma at p64
BA_COL = 132         # p64: x_last
BM_COL = 133         # unused
U64_COL = 134        # rows 0..63: upper-tri ones (64x64) + ones col 64;
                     # row 64 cols 0..63: ones (bcast lhs @ p64)
I64_COL = 199        # identity (64x64) [199:263)
BLOB_F = 263

_CACHE = {}

# HAM warmup experiment knobs (defaults off; enable only if A/B wins):
# WARM_N fp32 junk matmuls (64-col units) run during the input-DMA window to
# open the PE clock gate; BF16_FILL adds a 1-pass bf16 junk matmul per level,
# dependent on the level's nxtT copy, to keep duty high so the gate stays open.
WARM_N = 0
BF16_FILL = False
DUMMY_OUT_DMA = False  # early same-shaped DMA to warm the output queue family
LDW_FILL = True        # dependent junk bf16 LDWEIGHTS per squaring level: runs in
                       # the inter-level PE gap and keeps the pipe out of
                       # low-pstate (~50ns/level on the next first pass)

# dev knobs (ignored by graders): set TRACE=True before calling kernel() to
# capture an NTFF profile; the BassKernelResults lands in LAST_RESULT.
TRACE = False
LAST_RESULT = None



def _build_nc():
    nc = bacc.Bacc("TRN2", target_bir_lowering=False, debug=False)

    blob = nc.dram_tensor("blob", [K, BLOB_F], F32, kind="ExternalInput")
    y = nc.dram_tensor("y", [STEPS], F32, kind="ExternalOutput")

    with tile.TileContext(nc) as tc:
        with (
            tc.tile_pool(name="sb", bufs=1) as sb,
            tc.tile_pool(name="ps", bufs=2, space="PSUM") as ps,
        ):
            M = sb.tile([K, BLOB_F], F32, tag="M")
            nc.sync.dma_start(out=M[:], in_=blob[:])
            if DUMMY_OUT_DMA:
                scratch = nc.dram_tensor("scratch", [NCHUNK, P], F32)
                nc.sync.dma_start(out=scratch[:], in_=M[0:NCHUNK, 0:P])

            if WARM_N:
                junk = sb.tile([64, 128], F32, tag="junk")
                nc.vector.memset(junk[:], 0.5)   # DVE: runs right after entry
                pj = ps.tile([64, 64], F32, tag="pj", bufs=1)
                for _ in range(WARM_N):
                    nc.tensor.matmul(
                        pj[:], lhsT=junk[:, 0:64], rhs=junk[:, 64:128],
                        start=True, stop=True,
                    )
            pjb = None
            if BF16_FILL:
                pjb = ps.tile([64, 64], F32, tag="pjb", bufs=1)
                mbf = M[0:P, U64_COL : U64_COL + P].bitcast(mybir.dt.bfloat16)

            def ldw_fill(dep_ap):
                # junk bf16 stationary load (~60ns) whose source aliases
                # freshly copied data: runs in the inter-level PE gap and
                # keeps the pipe out of low-pstate, shaving ~50ns off the
                # next real matmul's first pass
                if not LDW_FILL:
                    return
                bf = dep_ap.bitcast(mybir.dt.bfloat16)
                n = min(64, bf.shape[-1] - 1)
                nc.tensor.ldweights(weights=bf[0 : min(64, bf.shape[0]), 1 : 1 + n])

            def bf_fill(dep_ap):
                if not BF16_FILL:
                    return
                bf = dep_ap.bitcast(mybir.dt.bfloat16)
                ncols = min(8, bf.shape[-1] // 2)
                nc.tensor.matmul(
                    pjb[0:64, 0:ncols],
                    lhsT=mbf[0:64, 1 : 2 * P : 2],
                    rhs=bf[0:64, 1 : 2 * ncols : 2],
                    start=True, stop=True,
                )

            cC = M[:, C_COL : C_COL + K]
            cT = M[:, CT_COL : CT_COL + K]
            u65 = M[0:P, U64_COL : U64_COL + P + 1]   # upper-tri + ones col
            i64 = M[0:P, I64_COL : I64_COL + P]
            ones_row64 = M[K - 1 : K, U64_COL : U64_COL + P]  # ones (1,64) @p64
            xl64 = M[K - 1 : K, BA_COL : BA_COL + 1]          # x_last @ p64

            # ---- power chain: C^2..C^64=G, then G^2..G^16 ------------------
            # (A@A)^T = A^T@A^T: out=lhsT.T@rhs gives M2=mm(MT, M), M2T=mm(M, MT)
            def square(a, aT, tag, need_plain=True):
                pa = ps.tile([K, K], F32, tag="psq")
                nxtT = sb.tile([K, K], F32, tag=f"{tag}T")
                nc.tensor.matmul(pa[:], lhsT=a[:], rhs=aT[:], start=True, stop=True)
                nc.scalar.copy(nxtT[:], pa[:])
                if not need_plain:
                    return None, nxtT
                pb = ps.tile([K, K], F32, tag="psq")
                nxt = sb.tile([K, K], F32, tag=tag)
                nc.tensor.matmul(pb[:], lhsT=aT[:], rhs=a[:], start=True, stop=True)
                nc.vector.tensor_copy(nxt[:], pb[:])
                bf_fill(nxtT[:])
                ldw_fill(nxtT[:])
                return nxt, nxtT

            powers = {}
            cur, curT = cC, cT
            for lvl in range(1, 10):          # lvl l holds C^(2^l): C^2..C^512
                cur, curT = square(cur, curT, f"p{lvl}")
                powers[lvl] = (cur, curT)

            # G = C^64 (lvl 6); G^2 = lvl 7; G^4 = lvl 8; G^8 = lvl 9
            GT = powers[6][1]
            G2T = powers[7][1]
            G4T = powers[8][1]
            G8, G8T = powers[9]

            # s0 = [d_0..d_63, c]: the state's constant lane carries
            # c = b_ar + b_ma, produced by the same subtract (the blob plants
            # b_ar / -b_ma at partition 64 of the diff columns); the C
            # skeleton has a structural 1 at [63,64] and [64,64].  Emitted
            # here (not at the top) so the DVE queue doesn't run it ahead of
            # the early power-chain copies; it's only needed for w1.
            s0 = sb.tile([K, 1], F32, tag="s0")
            nc.vector.tensor_sub(
                s0[:], M[:, XTA_COL : XTA_COL + 1], M[:, XTB_COL : XTB_COL + 1]
            )

            # ---- W doubling: W col j = s_{64(j+1)} -------------------------
            W = sb.tile([K, NCHUNK], F32, tag="W")

            def wcols(lhsT_ap, src_lo, src_n, dst_lo):
                pw = ps.tile([K, src_n], F32, tag="pw")
                nc.tensor.matmul(
                    pw[:], lhsT=lhsT_ap[:], rhs=W[:, src_lo : src_lo + src_n],
                    start=True, stop=True,
                )
                nc.vector.tensor_copy(W[:, dst_lo : dst_lo + src_n], pw[:])

            # w1 = G s0
            pw0 = ps.tile([K, 1], F32, tag="pw")
            nc.tensor.matmul(pw0[:], lhsT=GT[:], rhs=s0[:], start=True, stop=True)
            nc.vector.tensor_copy(W[:, 0:1], pw0[:])
            wcols(GT, 0, 1, 1)      # w2
            wcols(G2T, 0, 2, 2)     # w3 w4
            wcols(G4T, 0, 4, 4)     # w5..w8
            wcols(G8T, 0, 8, 8)     # w9..w16
            # G^16T (= C^1024 T) via T-only squaring of G^8
            _, G16T = square(G8, G8T, "p10", need_plain=False)
            wcols(G16T, 0, 16, 16)  # w17..w32

            # ---- cumsum: tri-matmuls (u65 row 64 = chunk sums) + scans ----
            # two PSUM banks so the first half's scan/copies aren't
            # bank-serialized against the second tri-matmul
            HN = NCHUNK // 2
            cum_a = ps.tile([K, HN], F32, tag="cum_a", bufs=1)
            cum_b = ps.tile([K, HN], F32, tag="cum_b", bufs=1)
            nc.tensor.matmul(cum_a[:], lhsT=u65, rhs=W[0:P, 0:HN],
                             start=True, stop=True)
            nc.tensor.matmul(cum_b[:], lhsT=u65, rhs=W[0:P, HN:NCHUNK],
                             start=True, stop=True)

            # ys copies pinned ahead of the scans in the DVE queue (ACT takes
            # the early half); the transpose below then hides under the scan.
            ys = sb.tile([P, NCHUNK], F32, tag="ys")
            with tc.high_priority():
                nc.scalar.copy(ys[:, 0:HN], cum_a[0:P, :])
                nc.vector.tensor_copy(ys[:, HN:NCHUNK], cum_b[0:P, :])

            # X[64, 0:32] = exclusive chunk offsets, x_last folded in; the
            # first scan half runs under tri-b, the second chains off it
            X = sb.tile([K, NCHUNK + 1], F32, tag="X")
            nc.vector.tensor_copy(X[K - 1 : K, 0:1], xl64)
            nc.vector.tensor_tensor_scan(
                out=X[K - 1 : K, 1 : HN + 1],
                data0=cum_a[K - 1 : K, :],
                data1=M[K - 1 : K, 0:HN],  # ignored (op1=bypass); SBUF
                initial=xl64,
                op0=mybir.AluOpType.add, op1=mybir.AluOpType.bypass,
            )
            nc.vector.tensor_tensor_scan(
                out=X[K - 1 : K, HN + 1 : NCHUNK + 1],
                data0=cum_b[K - 1 : K, :],
                data1=M[K - 1 : K, 0:HN],  # ignored (op1=bypass); SBUF
                initial=X[K - 1 : K, HN : HN + 1],
                op0=mybir.AluOpType.add, op1=mybir.AluOpType.bypass,
            )
            # transpose first (its input is ready before the scan finishes),
            # then accumulate the chunk-offset broadcast on top
            yt = ps.tile([NCHUNK, P], F32, tag="yt", bufs=1)
            nc.tensor.matmul(
                yt[:], lhsT=ys[:], rhs=i64, is_transpose=True,
                start=True, stop=False,
            )
            nc.tensor.matmul(
                yt[:], lhsT=X[K - 1 : K, 0:NCHUNK], rhs=ones_row64,
                start=False, stop=True,
            )
            yts = sb.tile([NCHUNK, P], F32, tag="yts")
            nc.vector.tensor_copy(yts[:, 0:40], yt[:, 0:40])
            nc.scalar.copy(yts[:, 40:P], yt[:, 40:P])
            nc.sync.dma_start(
                out=y[:].rearrange("(k i) -> k i", i=P), in_=yts[:]
            )

    nc.compile()
    return nc


def _make_blob(x, w_ar, b_ar, b_ma):
    """Pack inputs + structural constants into one DMA blob (layout only)."""
    blob = np.zeros((K, BLOB_F), np.float32)
    # C skeleton (oldest-first state, const lane carries c): s_t[i] =
    # s_{t-1}[i+1] for i<63, row 63 = [w_ar | 1], C[64,64]=1
    Cm = blob[:, C_COL : C_COL + K]
    for i in range(P - 1):
        Cm[i, i + 1] = 1.0
    Cm[P - 1, 0:P] = w_ar
    Cm[P - 1, P] = 1.0
    Cm[P, P] = 1.0
    blob[:, CT_COL : CT_COL + K] = Cm.T
    xt = np.asarray(x[0, -(P + 1) :, 0], np.float32)
    blob[0:P, XTA_COL] = xt[1 : P + 1]
    blob[0:P, XTB_COL] = xt[0:P]
    blob[P, XTA_COL] = b_ar            # sub yields c = b_ar + b_ma at p64
    blob[P, XTB_COL] = -b_ma
    blob[P, BA_COL] = xt[P]            # x_last @ p64 (scan initial)
    U = blob[0:P, U64_COL : U64_COL + P]
    U[np.triu_indices(P)] = 1.0        # U[j,i]=1 iff j<=i
    blob[0:P, U64_COL + P] = 1.0       # u65 ones col -> cum row 64 = sums
    blob[P, U64_COL : U64_COL + P] = 1.0  # ones row @ p64 (bcast lhsT)
    blob[0:P, I64_COL : I64_COL + P] = np.eye(P, dtype=np.float32)
    return blob


def kernel(x, w_ar, b_ar, b_ma, steps, w_ma=None, **_unused):
    assert int(steps) == STEPS, f"kernel compiled for steps={STEPS}, got {steps}"
    x = np.asarray(x, np.float32)
    assert x.shape[1] >= P + 1

    if "nc" not in _CACHE:
        _CACHE["nc"] = _build_nc()
    nc = _CACHE["nc"]

    blob = _make_blob(
        x,
        np.asarray(w_ar, np.float32),
        np.float32(np.asarray(b_ar, np.float32)),
        np.float32(np.asarray(b_ma, np.float32)),
    )
    res = run_bass_kernel_spmd(
        nc,
        [{"blob": blob} for _ in range(N_CORES)],
        core_ids=list(range(N_CORES)),
        trace=TRACE,
    )
    global LAST_RESULT
    LAST_RESULT = res
    return res.results[0]["y"].reshape(1, STEPS, 1)
